# revision 2
# baseline (speedup 1.0000x reference)
"""Trainium2 Bass kernel v2 for nn_Diff_prop_18425409699925 (GNN message passing).

Math (per batch element b, x = local_feat[b] reshaped to [n=1024, c=256]):
  rno_i = 1/||x_i||
  A     = (x x^T) * rno_i * rno_j          (cosine similarity)
  G     = exp(5*A), diagonal zeroed
  den_i = rowsum(G)  (= SP_i - e^5 where SP includes the diagonal)
  py    = G @ x                            (G symmetric -> computed c-major)
  diff  = x W^T - (py/den) W^T ; y = LeakyReLU(LN(diff))
LN is scale-invariant, so  y = LeakyReLU(LN(den*x W^T - py W^T)) exactly
(for the trivial-LN problem instance; gamma/beta/b_aff handled in fallback).

v2 pipeline per core:
  - dedicated >=3.5us PE warmup burst on memset tiles (HAM to 2.4GHz before
    real matmuls; the v1 warmup was too sparse and PE ran cold to 27.8us)
  - S row-blocks from a DMA'd fp8 c-major copy of raw x (no xn, no PE
    transposes, no xnT copies): matmuls start as soon as the DMA lands
  - rno broadcast RBC [128,1024] via one small PE transpose + 8 rank-1
    matmuls; tmp_k = S_k * RBC on DVE feeds exp(scale=5*rno_i) on ScalarE
  - pyT accumulated c-major from Xb (row-major fp8) and G8 (fp8), as v1
  - tail: den broadcast DBC the same way, t = xtb*DBC, qT = t - pyT (STT
    reading pyT PSUM directly), D = qT @ W^T (16 matmuls), bn_stats/aggr,
    fused Prelu finisher, paired output DMA.
"""

import os
import sys

import numpy as np

for _p in ("/opt/trn_rl_repo",):
    if os.path.isdir(_p) and _p not in sys.path:
        sys.path.insert(0, _p)

import ml_dtypes

import concourse.bacc as bacc
import concourse.bass as bass
import concourse.tile as tile
from concourse import mybir
from concourse.bass_utils import run_bass_kernel_spmd

B, T, NN, C = 8, 16, 64, 256
N = T * NN            # 1024 nodes per batch element
P = 128               # partitions
NT = N // P           # 8 n-tiles
CT = C // P           # 2 c-tiles
F32 = mybir.dt.float32
BF16 = mybir.dt.bfloat16
F8 = mybir.dt.float8e4
I32 = mybir.dt.int32
TS = bass.ts
DR = mybir.MatmulPerfMode.DoubleRow

LN_EPS = 1e-5
LEAK = 0.01
E5 = 148.4131591025766          # exp(5): diagonal of exp(5*A) pre-zeroing
MAGIC = float(0x5F3759DF)       # fp32 rsqrt seed constant, as a float


def _emit_rsqrt(nc, sb, out, in_, tmp_tags, n, steps=2):
    """out[:, :n] = 1/sqrt(in_[:, :n]) entirely on DVE (magic seed + Newton
    steps). in_/out fp32 [P, n]. Avoids a second ScalarE table set."""
    mult = mybir.AluOpType.mult
    add = mybir.AluOpType.add
    f = sb.tile([P, n], F32, tag=tmp_tags[0], name="rsq_f")
    ii = sb.tile([P, n], I32, tag=tmp_tags[1], name="rsq_i")
    a = sb.tile([P, n], F32, tag=tmp_tags[2], name="rsq_a")
    c = sb.tile([P, n], F32, tag=tmp_tags[3], name="rsq_c")
    nc.vector.tensor_copy(f[:], in_.bitcast(I32))          # int -> float value
    nc.vector.tensor_scalar(out=f[:], in0=f[:], scalar1=-0.5, scalar2=MAGIC,
                            op0=mult, op1=add)
    nc.vector.tensor_copy(ii[:], f[:])                     # float -> int round
    y = ii.bitcast(F32)
    for _ in range(steps):
        nc.vector.tensor_tensor(out=a[:], in0=y[:], in1=y[:], op=mult)
        nc.vector.scalar_tensor_tensor(out=a[:], in0=a[:], scalar=-0.5,
                                       in1=in_, op0=mult, op1=mult)
        nc.vector.tensor_scalar(out=c[:], in0=a[:], scalar1=1.5, scalar2=None,
                                op0=add)
        nc.vector.tensor_tensor(out=y[:], in0=y[:], in1=c[:], op=mult)
    nc.vector.tensor_copy(out, y[:])


def _build_program(diag_one, ln_trivial, cfg):
    nc = bacc.Bacc("TRN2", target_bir_lowering=False, debug=False)

    xb_d = nc.declare_dram_parameter("xb", [P, NT, C], F8, isOutput=False)
    xf_d = nc.declare_dram_parameter("xf", [P, CT, N], F8, isOutput=False)
    xt_d = nc.declare_dram_parameter("xt", [CT, P, N], BF16, isOutput=False)
    wtb_d = nc.declare_dram_parameter("wtb", [CT, P, C], BF16, isOutput=False)
    ng_d = nc.declare_dram_parameter("ng", [P, P], BF16, isOutput=False)
    aux_d = nc.declare_dram_parameter("aux", [P, 8], F32, isOutput=False)
    if not ln_trivial:
        auxv_d = nc.declare_dram_parameter("auxv", [1, 3 * C], F32,
                                           isOutput=False)
    else:
        auxv_d = None
    y_d = nc.declare_dram_parameter("y", [P, NT, C], BF16, isOutput=True)

    with tile.TileContext(nc) as tc:
        _emit(nc, tc, xb_d, xf_d, xt_d, wtb_d, ng_d, aux_d, auxv_d, y_d,
              diag_one, ln_trivial, cfg)
    nc.finalize()
    return nc


def _emit(nc, tc, xb_d, xf_d, xt_d, wtb_d, ng_d, aux_d, auxv_d, y_d,
          diag_one, ln_trivial, cfg):
    from contextlib import ExitStack

    mult = mybir.AluOpType.mult
    add = mybir.AluOpType.add
    subtract = mybir.AluOpType.subtract
    bypass = mybir.AluOpType.bypass
    amax = mybir.AluOpType.max
    AF = mybir.ActivationFunctionType

    use_prelu = cfg["prelu"]
    n_warm = cfg["warm_mm"]

    with ExitStack() as ctx:
        sb = ctx.enter_context(tc.tile_pool(name="sb", bufs=1))
        ps = ctx.enter_context(tc.tile_pool(name="ps", bufs=1, space="PSUM"))

        # ---------------- SBUF tiles ----------------
        Xb = sb.tile([P, NT, C], F8, tag="Xb", name="Xb")
        xf = sb.tile([P, CT, N], F8, tag="xf", name="xf")
        xtb = sb.tile([P, CT, N], BF16, tag="xtb", name="xtb")
        wtb = sb.tile([P, CT, C], BF16, tag="wtb", name="wtb")
        ng = sb.tile([P, P], BF16, tag="ng", name="ng")
        aux = sb.tile([P, 8], F32, tag="aux", name="aux")
        G8 = sb.tile([P, NT, N], F8, tag="G8", name="G8")
        Y = sb.tile([P, NT, C], BF16, tag="Y", name="Y")
        ones1 = sb.tile([1, P], BF16, tag="ones1", name="ones1")

        SS = sb.tile([P, NT], F32, tag="SS", name="SS")
        RNO = sb.tile([P, NT], F32, tag="RNO", name="RNO")
        RNO5 = sb.tile([P, NT], F32, tag="RNO5", name="RNO5")
        RNOb = sb.tile([P, NT], BF16, tag="RNOb", name="RNOb")
        rno8 = sb.tile([NT, P], BF16, tag="rno8", name="rno8")
        RBC = sb.tile([P, N], BF16, tag="RBC", name="RBC")
        SP = sb.tile([P, NT], F32, tag="SP", name="SP")
        SPDb = sb.tile([P, NT], BF16, tag="SPDb", name="SPDb")
        spd8 = sb.tile([NT, P], BF16, tag="spd8", name="spd8")
        DBC = sb.tile([P, N], BF16, tag="DBC", name="DBC")
        tmid = sb.tile([P, CT, N], BF16, tag="tmid", name="tmid")
        qT = sb.tile([P, CT, N], BF16, tag="qT", name="qT")

        BST = sb.tile([P, NT, 6], F32, tag="BST", name="BST")
        MV = sb.tile([P, NT, 2], F32, tag="MV", name="MV")
        VPE = sb.tile([P, NT], F32, tag="VPE", name="VPE")
        RSTD = sb.tile([P, NT], F32, tag="RSTD", name="RSTD")
        NB = sb.tile([P, NT], F32, tag="NB", name="NB")
        warm1 = sb.tile([P, 1], F32, tag="warm1", name="warm1")
        wl = sb.tile([P, P], BF16, tag="wl", name="wl")
        wr = sb.tile([P, 512], BF16, tag="wr", name="wr")

        if not diag_one:
            wdg = sb.tile([P, CT], F32, tag="wdg", name="wdg")
            xfs = sb.tile([P, CT, N], F8, tag="xfs", name="xfs")
        if not ln_trivial:
            auxv = sb.tile([1, 3 * C], F32, tag="auxv", name="auxv")
            brow = sb.tile([1, C], BF16, tag="brow", name="brow")
            gb = sb.tile([P, C], F32, tag="gb", name="gb")
            bb = sb.tile([P, C], F32, tag="bb", name="bb")

        # ------- loads: Xb first (squares chain), then xf (S matmuls) on
        # the sync HWDGE queue; xtb/wtb/ng on gpsimd SWDGE (needed later).
        for q in range(4):
            nc.sync.dma_start(Xb[:, TS(q, 2), :], xb_d[:, TS(q, 2), :])
        for q in range(2):
            nc.sync.dma_start(xf[:, q, :], xf_d[:, q, :])
        nc.gpsimd.dma_start(ng[:], ng_d[:])
        nc.gpsimd.dma_start(aux[:], aux_d[:])
        nc.gpsimd.dma_start(xtb[:], xt_d[:].rearrange("c p n -> p c n"))
        nc.gpsimd.dma_start(wtb[:], wtb_d[:].rearrange("c p d -> p c d"))
        if not ln_trivial:
            nc.gpsimd.dma_start(auxv[:], auxv_d[:])

        # Trigger the single ScalarE table load (exp set) immediately.
        nc.vector.memset(warm1[:], 0.0)
        wscr = sb.tile([P, 1], F32, tag="wscr", name="wscr")
        nc.scalar.activation(wscr[:], warm1[:], AF.Exp)

        # ---------------- PE warmup: dense burst on memset tiles ----------
        # HAM needs ~3.4us of sustained PE busy to un-throttle 1.2->2.4GHz.
        nc.vector.memset(wl[:], 0.0)
        nc.vector.memset(wr[:], 0.0)
        pwarm = ps.tile([P, 512], F32, tag="PY0", name="pwarm")
        for w in range(n_warm):
            nc.tensor.matmul(pwarm[:], wl[:], wr[:], start=True, stop=True)

        nc.vector.memset(ones1[:], 1.0)
        idb = sb.tile([P, P], BF16, tag="idb", name="idb")
        nc.vector.tensor_scalar(out=idb[:], in0=ng[:], scalar1=-1.0,
                                scalar2=1.0, op0=mult, op1=add)

        if not diag_one:
            # aux[:, 2:4] carries diag(W_adj) rearranged [P, CT]
            nc.vector.tensor_copy(wdg[:], aux[:, 2:4])
            for cc in range(CT):
                nc.vector.tensor_scalar(
                    out=xfs[:, cc, :], in0=xf[:, cc, :],
                    scalar1=wdg[:, cc:cc + 1], scalar2=None, op0=mult)
            lhs_xf = xfs
        else:
            lhs_xf = xf
        if not ln_trivial:
            nc.vector.tensor_copy(brow[:], auxv[:, 0:C])

        # ---------------- row norms -> RNO / RNO5 / RNOb ----------------
        for i in range(NT):
            sqs = sb.tile([P, C], F32, tag="sqs", bufs=2, name="sqs")
            nc.vector.scalar_tensor_tensor(
                out=sqs[:], in0=Xb[:, i, :], scalar=1.0, in1=Xb[:, i, :],
                op0=bypass, op1=mult, accum_out=SS[:, i:i + 1])
        _emit_rsqrt(nc, sb, RNO[:], SS[:], ("rsA", "rsB", "rsC", "rsD"), NT)
        nc.vector.tensor_scalar(out=RNO5[:], in0=RNO[:], scalar1=5.0,
                                scalar2=None, op0=mult)
        nc.vector.tensor_copy(RNOb[:], RNO[:])

        # RNO -> [NT, P] via PE transpose, then 8 rank-1 matmuls broadcast
        # rno to RBC [P, N] (column j holds rno_j).  These are emitted on the
        # PE queue BEFORE the S matmuls: RNO is ready (~9.8us) before xf's
        # DMA lands, so they cost no PE stall, and RBC gates the exp chain.
        ptrR = ps.tile([NT, P], BF16, tag="PY0", name="ptrR")
        nc.tensor.transpose(ptrR[:], RNOb[:], idb[:])
        nc.vector.tensor_copy(rno8[:], ptrR[:])
        # [8,128] -> [1,1024] row via SBUF-to-SBUF DMA (matmul rhs must sit
        # at base partition 0), then 2 rank-1 matmuls broadcast the row.
        rnoT = sb.tile([1, N], BF16, tag="rnoT", name="rnoT")
        nc.sync.dma_start(rnoT[0:1, :], rno8[:])
        pRBC = ps.tile([P, N], F32, tag="PY0", name="pRBC")
        for j in range(2):
            nc.tensor.matmul(pRBC[:, TS(j, 512)], ones1[:],
                             rnoT[0:1, TS(j, 512)], start=True, stop=True)
        # PSUM->SBUF bf16 copies on ScalarE (idle until exp0; keeps DVE free)
        for h in range(2):
            nc.scalar.activation(RBC[:, TS(h, 512)], pRBC[:, TS(h, 512)],
                                 AF.Copy)

        # ---------------- S matmuls + exp chain + pyT ----------------
        # pa_k = x x^T row-block k (raw, fp8 DoubleRow over c=256)
        pa_tiles = {}

        def emit_s_block(k):
            pa_k = ps.tile([P, N], F32, tag=f"A{k % 2}", name=f"pa{k}")
            for j in range(2):
                nc.tensor.matmul(
                    pa_k[:, TS(j, 512)],
                    lhs_xf[:, :, TS(k, P)],
                    xf[:, :, TS(j, 512)],
                    start=True, stop=True, perf_mode=DR)
            return pa_k

        for k in range(2):
            pa_tiles[k] = emit_s_block(k)

        pyT = [ps.tile([P, N], F32, tag=f"PY{k}", name=f"pyT{k}")
               for k in range(CT)]

        for k in range(NT):
            # tmp_k = pa_k * rno_j  (column scale via RBC), bf16
            tmp = sb.tile([P, N], BF16, tag="tmp", bufs=2, name="tmp")
            for h in range(2):
                nc.vector.tensor_tensor(
                    out=tmp[:, TS(h, 512)], in0=pa_tiles[k][:, TS(h, 512)],
                    in1=RBC[:, TS(h, 512)], op=mult)
            # G8_k = exp(5*rno_i * tmp_k), rowsum accumulated into SP
            nc.scalar.activation(G8[:, k, :], tmp[:], AF.Exp,
                                 scale=RNO5[:, k:k + 1],
                                 accum_out=SP[:, k:k + 1])
            pa_tiles.pop(k)
            # zero the diagonal block of G (it is ~e^5)
            nc.vector.tensor_tensor(out=G8[:, k, TS(k, P)],
                                    in0=G8[:, k, TS(k, P)], in1=ng[:], op=mult)
            if k + 2 < NT:
                pa_tiles[k + 2] = emit_s_block(k + 2)
            if k % 2 == 1:
                # pyT += X_pair^T @ G8_pair  (fp8 DoubleRow over the 2 blocks)
                kp = k // 2
                for cc in range(CT):
                    for j in range(2):
                        nc.tensor.matmul(
                            pyT[cc][:, TS(j, 512)],
                            Xb[:, k - 1:k + 1, TS(cc, P)],
                            G8[:, k - 1:k + 1, TS(j, 512)],
                            start=(kp == 0), stop=(kp == NT // 2 - 1),
                            perf_mode=DR)

        # ---------------- tail: den broadcast, qT, D = qT @ W^T -----------
        # SPDb = den = SP - e^5 (removes the zeroed diagonal term), bf16
        nc.vector.tensor_scalar(out=SPDb[:], in0=SP[:], scalar1=-E5,
                                scalar2=None, op0=add)
        ptrS = ps.tile([NT, P], BF16, tag="A1", name="ptrS")
        nc.tensor.transpose(ptrS[:], SPDb[:], idb[:])
        nc.vector.tensor_copy(spd8[:], ptrS[:])
        spdT = sb.tile([1, N], BF16, tag="spdT", name="spdT")
        nc.sync.dma_start(spdT[0:1, :], spd8[:])
        pDBC = ps.tile([P, N], F32, tag="A1", name="pDBC")
        for j in range(2):
            nc.tensor.matmul(pDBC[:, TS(j, 512)], ones1[:],
                             spdT[0:1, TS(j, 512)], start=True, stop=True)
        for h in range(2):
            nc.scalar.activation(DBC[:, TS(h, 512)], pDBC[:, TS(h, 512)],
                                 AF.Copy)

        # t = xtb * DBC ; qT = t - pyT   (per j-half so D matmuls start on
        # the first half while the second is still on DVE)
        for h in range(2):
            for cc in range(CT):
                nc.vector.tensor_tensor(
                    out=tmid[:, cc, TS(h, 512)], in0=xtb[:, cc, TS(h, 512)],
                    in1=DBC[:, TS(h, 512)], op=mult)
            for cc in range(CT):
                nc.vector.scalar_tensor_tensor(
                    out=qT[:, cc, TS(h, 512)], in0=pyT[cc][:, TS(h, 512)],
                    scalar=-1.0, in1=tmid[:, cc, TS(h, 512)],
                    op0=mult, op1=add)

        # D_i = q_i @ W^T  (bf16, accumulated over the two c chunks).
        # Paired by PSUM bank: both i of a 512-col bank get their matmuls
        # emitted before either bn_stats, so DVE never reads a bank the PE
        # is still writing (PSUM collision is fatal in HW).
        pDA = ps.tile([P, N], F32, tag="A0", name="pDA")
        pDB = ps.tile([P, N], F32, tag="A1", name="pDB")
        pD = [pDA[:, TS(i, C)] for i in range(4)] + \
             [pDB[:, TS(i, C)] for i in range(4)]
        half = NT // 2
        for pp in range(NT // 2):
            for i in (2 * pp, 2 * pp + 1):
                for cc in range(CT):
                    last = (cc == CT - 1) and ln_trivial
                    nc.tensor.matmul(pD[i], qT[:, cc, TS(i, P)],
                                     wtb[:, cc, :], start=(cc == 0), stop=last)
                if not ln_trivial:
                    # + den_i * b_aff  (rank-1, spd8 row slice x brow)
                    nc.tensor.matmul(pD[i], spd8[i:i + 1, :], brow[:],
                                     start=False, stop=True)
            for i in (2 * pp, 2 * pp + 1):
                nc.vector.bn_stats(BST[:, i, :], pD[i])
                nc.vector.bn_aggr(MV[:, i, :], BST[:, i, :])

        def emit_finish(h):
            hs = slice(h * half, (h + 1) * half)
            nc.vector.tensor_scalar(out=VPE[:, hs], in0=MV[:, hs, 1],
                                    scalar1=LN_EPS, scalar2=None, op0=add)
            _emit_rsqrt(nc, sb, RSTD[:, hs], VPE[:, hs],
                        ("rsA", "rsB", "rsC", "rsD"), half, steps=2)
            nc.vector.scalar_tensor_tensor(out=NB[:, hs], in0=MV[:, hs, 0],
                                           scalar=-1.0, in1=RSTD[:, hs],
                                           op0=mult, op1=mult)
            for i in range(h * half, (h + 1) * half):
                if ln_trivial and use_prelu and i % 4 != 3:
                    nc.scalar.activation(Y[:, i, :], pD[i], AF.Prelu,
                                         bias=NB[:, i:i + 1],
                                         scale=RSTD[:, i:i + 1], alpha=LEAK)
                else:
                    t = sb.tile([P, C], BF16, tag="fin", bufs=2, name="fin")
                    nc.vector.tensor_scalar(
                        out=t[:], in0=pD[i], scalar1=RSTD[:, i:i + 1],
                        scalar2=NB[:, i:i + 1], op0=mult, op1=add)
                    if not ln_trivial:
                        u2 = sb.tile([P, C], F32, tag="fin2", bufs=2,
                                     name="fin2")
                        nc.vector.scalar_tensor_tensor(
                            out=u2[:], in0=t[:], scalar=1.0, in1=gb[:],
                            op0=bypass, op1=mult)
                        nc.vector.scalar_tensor_tensor(
                            out=t[:], in0=u2[:], scalar=1.0, in1=bb[:],
                            op0=bypass, op1=add)
                    nc.vector.scalar_tensor_tensor(
                        out=Y[:, i, :], in0=t[:], scalar=LEAK, in1=t[:],
                        op0=mult, op1=amax)
                if i % 2 == 1:
                    nc.sync.dma_start(y_d[:, i - 1:i + 1, :],
                                      Y[:, i - 1:i + 1, :])

        if not ln_trivial:
            # gamma/beta broadcast to [P, C] via rank-1 PE matmuls
            pgb = ps.tile([P, C], F32, tag="PY0", name="pgb")
            pbb = ps.tile([P, C], F32, tag="PY1", name="pbb")
            gbrow = sb.tile([1, C], BF16, tag="gbrow", name="gbrow")
            bbrow = sb.tile([1, C], BF16, tag="bbrow", name="bbrow")
            nc.vector.tensor_copy(gbrow[:], auxv[:, C:2 * C])
            nc.vector.tensor_copy(bbrow[:], auxv[:, 2 * C:3 * C])
            nc.tensor.matmul(pgb[:], ones1[:], gbrow[:], start=True, stop=True)
            nc.tensor.matmul(pbb[:], ones1[:], bbrow[:], start=True, stop=True)
            nc.vector.tensor_copy(gb[:], pgb[:])
            nc.vector.tensor_copy(bb[:], pbb[:])

        emit_finish(0)
        emit_finish(1)


_PROGRAM_CACHE = {}
last_results = None


def _cfg():
    return {
        "prelu": bool(int(os.environ.get("KERNEL_PRELU", "1"))),
        "warm_mm": int(os.environ.get("KERNEL_WARM_MM", "9")),
    }


def _get_program(diag_one=True, ln_trivial=True):
    cfg = _cfg()
    key = (diag_one, ln_trivial, tuple(sorted(cfg.items())))
    if key not in _PROGRAM_CACHE:
        _PROGRAM_CACHE[key] = _build_program(diag_one, ln_trivial, cfg)
    return _PROGRAM_CACHE[key]


def _prep_inputs(local_feat, W_adj, W_aff, b_aff, ln_gamma, ln_beta):
    x_full = np.asarray(local_feat, np.float32).reshape(B, N, C)
    diag = np.ascontiguousarray(np.diagonal(np.asarray(W_adj, np.float32)))
    diag_one = bool(np.all(diag == 1.0))
    g = np.asarray(ln_gamma, np.float32).ravel()
    be = np.asarray(ln_beta, np.float32).ravel()
    b = np.asarray(b_aff, np.float32).ravel()
    ln_trivial = bool(np.all(g == 1.0) and np.all(be == 0.0)
                      and np.all(b == 0.0))

    bf = ml_dtypes.bfloat16
    wt = np.ascontiguousarray(
        np.asarray(W_aff, np.float32).T.reshape(CT, P, C)).astype(bf)
    ng = (1.0 - np.eye(P, dtype=np.float32)).astype(bf)
    aux = np.zeros((P, 8), np.float32)
    if not diag_one:
        aux[:, 2:4] = diag.reshape(CT, P).T
    auxv = None
    if not ln_trivial:
        auxv = np.concatenate([b, g, be]).reshape(1, 3 * C).astype(np.float32)

    f8 = ml_dtypes.float8_e4m3
    in_maps = []
    for bb in range(B):
        x = x_full[bb]
        xb = np.ascontiguousarray(
            x.reshape(NT, P, C).transpose(1, 0, 2)).astype(f8)
        xf = np.ascontiguousarray(
            x.T.reshape(CT, P, N).transpose(1, 0, 2)).astype(f8)
        xt = np.ascontiguousarray(x.T.reshape(CT, P, N)).astype(bf)
        m = {"xb": xb, "xf": xf, "xt": xt, "wtb": wt, "ng": ng, "aux": aux}
        if auxv is not None:
            m["auxv"] = auxv
        in_maps.append(m)
    return in_maps, diag_one, ln_trivial


def kernel(local_feat, global_feat, pos, W_adj, W_aff, b_aff, ln_gamma,
           ln_beta, **_unused):
    global last_results
    in_maps, diag_one, ln_trivial = _prep_inputs(
        local_feat, W_adj, W_aff, b_aff, ln_gamma, ln_beta)
    nc = _get_program(diag_one, ln_trivial)
    trace = bool(int(os.environ.get("KERNEL_TRACE", "0")))
    res = run_bass_kernel_spmd(nc, in_maps, list(range(B)), trace=trace)
    last_results = res
    out = np.empty((B, N, C), np.float32)
    for bb in range(B):
        yb = np.asarray(res.results[bb]["y"]).astype(np.float32)  # [P, NT, C]
        out[bb] = yb.transpose(1, 0, 2).reshape(N, C)
    return out.reshape(B, T, NN, C)


# revision 4
# speedup vs baseline: 1.1076x; 1.1076x over previous
"""Trainium2 Bass kernel v3 for nn_Diff_prop_18425409699925 (GNN message passing).

Math (per batch element, x = local_feat[b] reshaped to [n=1024, c=256]):
  rno_i = 1/||x_i||;  xn = x*rno;  A = xn xn^T;  G = exp(5A), diag zeroed
  den_i = rowsum(G) = SP_i - e^5;  py = G @ x  (G symmetric -> c-major)
  diff  = x W^T - (py/den) W^T;  y = LeakyReLU(LN(diff))
LN is scale-invariant => y = LeakyReLU(LN(den * xW^T - pyW^T)) exactly:
the 1/den division disappears and den enters only as a per-partition scalar.

v3 pipeline (fixes measured v2 sins):
  - input DMA split across sync + gpsimd queues (v2: one queue, x landed
    ~15us and everything chained after it)
  - >=5us dense PE warmup burst on memset tiles (HAM 1.2->2.4GHz)
  - rno column-broadcast built from 8 [128,128] per-block row-broadcasts
    (DVE tensor_scalar) + 8 PE transposes into one bf16 PSUM tile: no
    SBUF->SBUF DMA hop, no rank-1 matmuls, ~2us after RNO
  - normalization folded ONCE into xn8 = xf * RBC (fp8); the A matmuls and
    exp(scale=5.0, PSUM f32 input) then run exactly like the v1 baseline
    (v2's per-block tmp multiply made the exp chain DVE-paced)
  - tail: pd = xW^T and pu = (py)W^T matmul streams, then one dual-PSUM
    STT per row block D_i = den_i*pd_i - pu_i -> SBUF f32; bn_stats/aggr
    on SBUF; fused Prelu finisher; paired output DMA.  No den broadcast.
"""

import os
import sys

import numpy as np

for _p in ("/opt/trn_rl_repo",):
    if os.path.isdir(_p) and _p not in sys.path:
        sys.path.insert(0, _p)

import ml_dtypes

import concourse.bacc as bacc
import concourse.bass as bass
import concourse.tile as tile
from concourse import mybir
from concourse.bass_utils import run_bass_kernel_spmd

B, T, NN, C = 8, 16, 64, 256
N = T * NN            # 1024 nodes per batch element
P = 128               # partitions
NT = N // P           # 8 n-tiles
CT = C // P           # 2 c-tiles
F32 = mybir.dt.float32
BF16 = mybir.dt.bfloat16
F8 = mybir.dt.float8e4
I32 = mybir.dt.int32
TS = bass.ts
DR = mybir.MatmulPerfMode.DoubleRow

LN_EPS = 1e-5
LEAK = 0.01
E5 = 148.4131591025766          # exp(5): diagonal of exp(5*A) pre-zeroing
MAGIC = float(0x5F3759DF)       # fp32 rsqrt seed constant, as a float


def _emit_rsqrt(nc, sb, out, in_, tmp_tags, n, steps=2):
    """out[:, :n] = 1/sqrt(in_[:, :n]) entirely on DVE (magic seed + Newton
    steps). in_/out fp32 [P, n]. Avoids a second ScalarE table set."""
    mult = mybir.AluOpType.mult
    add = mybir.AluOpType.add
    f = sb.tile([P, n], F32, tag=tmp_tags[0], name="rsq_f")
    ii = sb.tile([P, n], I32, tag=tmp_tags[1], name="rsq_i")
    a = sb.tile([P, n], F32, tag=tmp_tags[2], name="rsq_a")
    c = sb.tile([P, n], F32, tag=tmp_tags[3], name="rsq_c")
    nc.vector.tensor_copy(f[:], in_.bitcast(I32))          # int -> float value
    nc.vector.tensor_scalar(out=f[:], in0=f[:], scalar1=-0.5, scalar2=MAGIC,
                            op0=mult, op1=add)
    nc.vector.tensor_copy(ii[:], f[:])                     # float -> int round
    y = ii.bitcast(F32)
    for _ in range(steps):
        nc.vector.tensor_tensor(out=a[:], in0=y[:], in1=y[:], op=mult)
        nc.vector.scalar_tensor_tensor(out=a[:], in0=a[:], scalar=-0.5,
                                       in1=in_, op0=mult, op1=mult)
        nc.vector.tensor_scalar(out=c[:], in0=a[:], scalar1=1.5, scalar2=None,
                                op0=add)
        nc.vector.tensor_tensor(out=y[:], in0=y[:], in1=c[:], op=mult)
    nc.vector.tensor_copy(out, y[:])


def _build_program(diag_one, ln_trivial, cfg):
    nc = bacc.Bacc("TRN2", target_bir_lowering=False, debug=False)

    xb_d = nc.declare_dram_parameter("xb", [P, NT, C], F8, isOutput=False)
    xf_d = nc.declare_dram_parameter("xf", [P, CT, N], F8, isOutput=False)
    xt_d = nc.declare_dram_parameter("xt", [CT, P, N], BF16, isOutput=False)
    wtb_d = nc.declare_dram_parameter("wtb", [CT, P, C], BF16, isOutput=False)
    ng_d = nc.declare_dram_parameter("ng", [P, P], BF16, isOutput=False)
    aux_d = nc.declare_dram_parameter("aux", [P, 8], F32, isOutput=False)
    if not ln_trivial:
        auxv_d = nc.declare_dram_parameter("auxv", [1, 3 * C], F32,
                                           isOutput=False)
    else:
        auxv_d = None
    y_d = nc.declare_dram_parameter("y", [P, NT, C], BF16, isOutput=True)

    with tile.TileContext(nc) as tc:
        _emit(nc, tc, xb_d, xf_d, xt_d, wtb_d, ng_d, aux_d, auxv_d, y_d,
              diag_one, ln_trivial, cfg)
    nc.finalize()
    return nc


def _emit(nc, tc, xb_d, xf_d, xt_d, wtb_d, ng_d, aux_d, auxv_d, y_d,
          diag_one, ln_trivial, cfg):
    from contextlib import ExitStack

    mult = mybir.AluOpType.mult
    add = mybir.AluOpType.add
    subtract = mybir.AluOpType.subtract
    bypass = mybir.AluOpType.bypass
    amax = mybir.AluOpType.max
    AF = mybir.ActivationFunctionType

    use_prelu = cfg["prelu"]
    n_warm = cfg["warm_mm"]

    with ExitStack() as ctx:
        sb = ctx.enter_context(tc.tile_pool(name="sb", bufs=1))
        ps = ctx.enter_context(tc.tile_pool(name="ps", bufs=1, space="PSUM"))

        # ---------------- SBUF tiles ----------------
        Xb = sb.tile([P, NT, C], F8, tag="Xb", name="Xb")
        xf = sb.tile([P, CT, N], F8, tag="xf", name="xf")
        xn8 = sb.tile([P, CT, N], F8, tag="xn8", name="xn8")
        xtb = sb.tile([P, CT, N], BF16, tag="xtb", name="xtb")
        wtb = sb.tile([P, CT, C], BF16, tag="wtb", name="wtb")
        ng = sb.tile([P, P], BF16, tag="ng", name="ng")
        aux = sb.tile([P, 8], F32, tag="aux", name="aux")
        G8 = sb.tile([P, NT, N], F8, tag="G8", name="G8")
        spyT = [sb.tile([P, N], BF16, tag=f"spyT{k}", name=f"spyT{k}")
                for k in range(CT)]
        D = sb.tile([P, NT, C], F32, tag="D", name="D")
        Y = sb.tile([P, NT, C], BF16, tag="Y", name="Y")
        ones1 = sb.tile([1, P], BF16, tag="ones1", name="ones1")
        ON1 = sb.tile([P, P], BF16, tag="ON1", name="ON1")

        SS = sb.tile([P, NT], F32, tag="SS", name="SS")
        RNO = sb.tile([P, NT], F32, tag="RNO", name="RNO")
        RBC = sb.tile([P, N], BF16, tag="RBC", name="RBC")
        SP = sb.tile([P, NT], F32, tag="SP", name="SP")
        SPD = sb.tile([P, NT], F32, tag="SPD", name="SPD")

        BST = sb.tile([P, NT, 6], F32, tag="BST", name="BST")
        MV = sb.tile([P, NT, 2], F32, tag="MV", name="MV")
        VPE = sb.tile([P, NT], F32, tag="VPE", name="VPE")
        RSTD = sb.tile([P, NT], F32, tag="RSTD", name="RSTD")
        NB = sb.tile([P, NT], F32, tag="NB", name="NB")
        warm1 = sb.tile([P, 1], F32, tag="warm1", name="warm1")
        wl = sb.tile([P, P], BF16, tag="wl", name="wl")
        wr = sb.tile([P, 512], BF16, tag="wr", name="wr")

        if not diag_one:
            wdg = sb.tile([P, CT], F32, tag="wdg", name="wdg")
            xn8s = sb.tile([P, CT, N], F8, tag="xn8s", name="xn8s")
        if not ln_trivial:
            auxv = sb.tile([1, 3 * C], F32, tag="auxv", name="auxv")
            bb2 = sb.tile([P, C], F32, tag="bb2", name="bb2")
            gb = sb.tile([P, C], F32, tag="gb", name="gb")
            bb = sb.tile([P, C], F32, tag="bb", name="bb")

        # ------- loads split across both DMA queues so x lands ~2x sooner.
        # sync (HWDGE): Xb blocks 0-3, xf chunk 0, xtb
        # gpsimd (SWDGE): ng, aux, Xb blocks 4-7, xf chunk 1, wtb
        nc.sync.dma_start(Xb[:, 0:2, :], xb_d[:, 0:2, :])
        nc.sync.dma_start(Xb[:, 2:4, :], xb_d[:, 2:4, :])
        nc.sync.dma_start(xf[:, 0, :], xf_d[:, 0, :])
        nc.gpsimd.dma_start(ng[:], ng_d[:])
        nc.gpsimd.dma_start(aux[:], aux_d[:])
        nc.gpsimd.dma_start(Xb[:, 4:6, :], xb_d[:, 4:6, :])
        nc.gpsimd.dma_start(Xb[:, 6:8, :], xb_d[:, 6:8, :])
        nc.gpsimd.dma_start(xf[:, 1, :], xf_d[:, 1, :])
        nc.sync.dma_start(xtb[:], xt_d[:].rearrange("c p n -> p c n"))
        nc.gpsimd.dma_start(wtb[:], wtb_d[:].rearrange("c p d -> p c d"))
        if not ln_trivial:
            nc.gpsimd.dma_start(auxv[:], auxv_d[:])

        # Trigger the single ScalarE table load (exp set) immediately.
        nc.vector.memset(warm1[:], 0.0)
        wscr = sb.tile([P, 1], F32, tag="wscr", name="wscr")
        nc.scalar.activation(wscr[:], warm1[:], AF.Exp)

        # ---------------- PE warmup: dense burst on memset tiles ----------
        # HAM needs ~3.4us of sustained PE busy to un-throttle 1.2->2.4GHz.
        nc.vector.memset(wl[:], 0.0)
        nc.vector.memset(wr[:], 0.0)
        pwarm = ps.tile([P, 512], F32, tag="PY0", name="pwarm")
        for w in range(n_warm):
            nc.tensor.matmul(pwarm[:], wl[:], wr[:], start=True, stop=True)

        nc.vector.memset(ones1[:], 1.0)
        nc.vector.memset(ON1[:], 1.0)
        idb = sb.tile([P, P], BF16, tag="idb", name="idb")
        nc.vector.tensor_scalar(out=idb[:], in0=ng[:], scalar1=-1.0,
                                scalar2=1.0, op0=mult, op1=add)
        if not ln_trivial:
            # b_aff / gamma / beta broadcast to [P, C] on the idle Pool
            nc.gpsimd.partition_broadcast(bb2[:], auxv[0:1, 0:C])
            nc.gpsimd.partition_broadcast(gb[:], auxv[0:1, C:2 * C])
            nc.gpsimd.partition_broadcast(bb[:], auxv[0:1, 2 * C:3 * C])
        if not diag_one:
            nc.vector.tensor_copy(wdg[:], aux[:, 2:4])

        # ---------------- row norms -> RNO ----------------
        # squares split ScalarE/DVE so SS closes right after the last Xb
        # chunk lands (sq accumulates per block; blocks arrive two queues).
        for i in range(NT):
            if i % 2 == 0:
                sqa = sb.tile([P, C], F32, tag="sqa", bufs=2, name="sqa")
                nc.scalar.activation(sqa[:], Xb[:, i, :], AF.Square,
                                     accum_out=SS[:, i:i + 1])
            else:
                sqs = sb.tile([P, C], F32, tag="sqs", bufs=2, name="sqs")
                nc.vector.scalar_tensor_tensor(
                    out=sqs[:], in0=Xb[:, i, :], scalar=1.0, in1=Xb[:, i, :],
                    op0=bypass, op1=mult, accum_out=SS[:, i:i + 1])
        _emit_rsqrt(nc, sb, RNO[:], SS[:], ("rsA", "rsB", "rsC", "rsD"), NT)

        # ---------------- RBC: rno as columns, via 8 block transposes ------
        # RNOW_b = rno[:, b] broadcast across 128 cols (DVE), transposed by
        # the PE into RBC[:, 128b:128b+128].  bf16 PSUM tile = 1 bank.
        pRBC = ps.tile([P, N], BF16, tag="PY0", name="pRBC")
        for b in range(NT):
            RNOW = sb.tile([P, P], BF16, tag="RNOW", bufs=2, name="RNOW")
            nc.vector.tensor_scalar(out=RNOW[:], in0=ON1[:],
                                    scalar1=RNO[:, b:b + 1], scalar2=None,
                                    op0=mult)
            nc.tensor.transpose(pRBC[:, TS(b, P)], RNOW[:], idb[:])
        for h in range(2):
            nc.scalar.activation(RBC[:, TS(h, 512)], pRBC[:, TS(h, 512)],
                                 AF.Copy)

        # xn8 = xf * rno_col  (both matmul operands come from this one
        # normalized fp8 tensor).  j-half order so A_0 starts early.
        for h in range(2):
            for cc in range(CT):
                nc.vector.tensor_tensor(
                    out=xn8[:, cc, TS(h, 512)], in0=xf[:, cc, TS(h, 512)],
                    in1=RBC[:, TS(h, 512)], op=mult)
        if not diag_one:
            for cc in range(CT):
                nc.vector.tensor_scalar(
                    out=xn8s[:, cc, :], in0=xn8[:, cc, :],
                    scalar1=wdg[:, cc:cc + 1], scalar2=None, op0=mult)
            lhs_xn = xn8s
        else:
            lhs_xn = xn8

        # ---------------- A matmuls + exp chain + pyT ----------------
        pa_tiles = {}

        def emit_a_block(k):
            pa_k = ps.tile([P, N], F32, tag=f"A{k % 2}", name=f"pa{k}")
            for j in range(2):
                nc.tensor.matmul(
                    pa_k[:, TS(j, 512)],
                    lhs_xn[:, :, TS(k, P)],
                    xn8[:, :, TS(j, 512)],
                    start=True, stop=True, perf_mode=DR)
            return pa_k

        for k in range(2):
            pa_tiles[k] = emit_a_block(k)

        pyT = [ps.tile([P, N], F32, tag=f"PY{k}", name=f"pyT{k}")
               for k in range(CT)]

        for k in range(NT):
            nc.scalar.activation(G8[:, k, :], pa_tiles.pop(k)[:], AF.Exp,
                                 scale=5.0, accum_out=SP[:, k:k + 1])
            # zero the diagonal block of G (it is ~e^5)
            nc.vector.tensor_tensor(out=G8[:, k, TS(k, P)],
                                    in0=G8[:, k, TS(k, P)], in1=ng[:], op=mult)
            if k + 2 < NT:
                pa_tiles[k + 2] = emit_a_block(k + 2)
            if k % 2 == 1:
                # pyT += X_pair^T @ G8_pair  (fp8 DoubleRow over the 2 blocks)
                kp = k // 2
                for cc in range(CT):
                    for j in range(2):
                        nc.tensor.matmul(
                            pyT[cc][:, TS(j, 512)],
                            Xb[:, k - 1:k + 1, TS(cc, P)],
                            G8[:, k - 1:k + 1, TS(j, 512)],
                            start=(kp == 0), stop=(kp == NT // 2 - 1),
                            perf_mode=DR)

        # ---------------- tail ----------------
        # den_i = SP_i - e^5 (per-partition scalar; no broadcast needed)
        nc.vector.tensor_scalar(out=SPD[:], in0=SP[:], scalar1=-E5,
                                scalar2=None, op0=add)

        # spyT = pyT copied to SBUF (PE can't read PSUM); halves split
        # between ScalarE and DVE
        for cc in range(CT):
            for h in range(2):
                if cc == 0:
                    nc.scalar.activation(spyT[cc][:, TS(h, 512)],
                                         pyT[cc][:, TS(h, 512)], AF.Copy)
                else:
                    nc.vector.tensor_copy(spyT[cc][:, TS(h, 512)],
                                          pyT[cc][:, TS(h, 512)])

        # pd_i = x_i @ W^T (into the A banks freed by the exp chain), then
        # copied to SBUF (sd) so the final STT has only one PSUM operand.
        # Copies run pair-wise behind the next pair's matmuls (bank-safe)
        # and are hidden under the pu matmul stream.
        pdA = ps.tile([P, N], F32, tag="A0", name="pdA")
        pdB = ps.tile([P, N], F32, tag="A1", name="pdB")
        pd = [pdA[:, TS(i, C)] for i in range(4)] + \
             [pdB[:, TS(i, C)] for i in range(4)]
        sd = sb.tile([P, NT, C], F32, tag="sd", name="sd")
        for pp in range(NT // 2):
            for i in (2 * pp, 2 * pp + 1):
                for cc in range(CT):
                    nc.tensor.matmul(pd[i], xtb[:, cc, TS(i, P)],
                                     wtb[:, cc, :],
                                     start=(cc == 0), stop=(cc == CT - 1))
            for i in (2 * pp, 2 * pp + 1):
                if i % 2 == 0:
                    nc.scalar.activation(sd[:, i, :], pd[i], AF.Copy)
                else:
                    nc.vector.tensor_copy(sd[:, i, :], pd[i])

        # pu_i = py_i @ W^T (into the PY banks freed by the spyT copies),
        # then D_i = den_i*pd_i - pu_i via one dual-PSUM STT per block.
        # b_aff fallback: + den_i*b via a rank-1 matmul into pd first.
        puA = ps.tile([P, N], F32, tag="PY0", name="puA")
        puB = ps.tile([P, N], F32, tag="PY1", name="puB")
        pu = [puA[:, TS(i, C)] for i in range(4)] + \
             [puB[:, TS(i, C)] for i in range(4)]
        half = NT // 2

        for pp in range(NT // 2):
            for i in (2 * pp, 2 * pp + 1):
                for cc in range(CT):
                    nc.tensor.matmul(pu[i], spyT[cc][:, TS(i, P)],
                                     wtb[:, cc, :],
                                     start=(cc == 0), stop=(cc == CT - 1))
            for i in (2 * pp, 2 * pp + 1):
                nc.vector.scalar_tensor_tensor(
                    out=D[:, i, :], in0=sd[:, i, :], scalar=SPD[:, i:i + 1],
                    op0=mult, in1=pu[i], op1=subtract)
                if not ln_trivial:
                    # D_i += den_i * b_aff  (row-broadcast bb2, column den)
                    nc.vector.scalar_tensor_tensor(
                        out=D[:, i, :], in0=bb2[:], scalar=SPD[:, i:i + 1],
                        op0=mult, in1=D[:, i, :], op1=add)
                nc.vector.bn_stats(BST[:, i, :], D[:, i, :])
                nc.vector.bn_aggr(MV[:, i, :], BST[:, i, :])

        def emit_finish(h):
            hs = slice(h * half, (h + 1) * half)
            nc.vector.tensor_scalar(out=VPE[:, hs], in0=MV[:, hs, 1],
                                    scalar1=LN_EPS, scalar2=None, op0=add)
            _emit_rsqrt(nc, sb, RSTD[:, hs], VPE[:, hs],
                        ("rsA", "rsB", "rsC", "rsD"), half, steps=2)
            nc.vector.scalar_tensor_tensor(out=NB[:, hs], in0=MV[:, hs, 0],
                                           scalar=-1.0, in1=RSTD[:, hs],
                                           op0=mult, op1=mult)
            for i in range(h * half, (h + 1) * half):
                if ln_trivial and use_prelu and i % 4 != 3:
                    nc.scalar.activation(Y[:, i, :], D[:, i, :], AF.Prelu,
                                         bias=NB[:, i:i + 1],
                                         scale=RSTD[:, i:i + 1], alpha=LEAK)
                else:
                    t = sb.tile([P, C], BF16, tag="fin", bufs=2, name="fin")
                    nc.vector.tensor_scalar(
                        out=t[:], in0=D[:, i, :], scalar1=RSTD[:, i:i + 1],
                        scalar2=NB[:, i:i + 1], op0=mult, op1=add)
                    if not ln_trivial:
                        u2 = sb.tile([P, C], F32, tag="fin2", bufs=2,
                                     name="fin2")
                        nc.vector.scalar_tensor_tensor(
                            out=u2[:], in0=t[:], scalar=1.0, in1=gb[:],
                            op0=bypass, op1=mult)
                        nc.vector.scalar_tensor_tensor(
                            out=t[:], in0=u2[:], scalar=1.0, in1=bb[:],
                            op0=bypass, op1=add)
                    nc.vector.scalar_tensor_tensor(
                        out=Y[:, i, :], in0=t[:], scalar=LEAK, in1=t[:],
                        op0=mult, op1=amax)
                if i % 2 == 1:
                    nc.sync.dma_start(y_d[:, i - 1:i + 1, :],
                                      Y[:, i - 1:i + 1, :])

        emit_finish(0)
        emit_finish(1)


_PROGRAM_CACHE = {}
last_results = None


def _cfg():
    return {
        "prelu": bool(int(os.environ.get("KERNEL_PRELU", "1"))),
        "warm_mm": int(os.environ.get("KERNEL_WARM_MM", "13")),
    }


def _get_program(diag_one=True, ln_trivial=True):
    cfg = _cfg()
    key = (diag_one, ln_trivial, tuple(sorted(cfg.items())))
    if key not in _PROGRAM_CACHE:
        _PROGRAM_CACHE[key] = _build_program(diag_one, ln_trivial, cfg)
    return _PROGRAM_CACHE[key]


def _prep_inputs(local_feat, W_adj, W_aff, b_aff, ln_gamma, ln_beta):
    x_full = np.asarray(local_feat, np.float32).reshape(B, N, C)
    diag = np.ascontiguousarray(np.diagonal(np.asarray(W_adj, np.float32)))
    diag_one = bool(np.all(diag == 1.0))
    g = np.asarray(ln_gamma, np.float32).ravel()
    be = np.asarray(ln_beta, np.float32).ravel()
    b = np.asarray(b_aff, np.float32).ravel()
    ln_trivial = bool(np.all(g == 1.0) and np.all(be == 0.0)
                      and np.all(b == 0.0))

    bf = ml_dtypes.bfloat16
    wt = np.ascontiguousarray(
        np.asarray(W_aff, np.float32).T.reshape(CT, P, C)).astype(bf)
    ng = (1.0 - np.eye(P, dtype=np.float32)).astype(bf)
    aux = np.zeros((P, 8), np.float32)
    if not diag_one:
        aux[:, 2:4] = diag.reshape(CT, P).T
    auxv = None
    if not ln_trivial:
        auxv = np.concatenate([b, g, be]).reshape(1, 3 * C).astype(np.float32)

    f8 = ml_dtypes.float8_e4m3
    in_maps = []
    for bb in range(B):
        x = x_full[bb]
        xb = np.ascontiguousarray(
            x.reshape(NT, P, C).transpose(1, 0, 2)).astype(f8)
        xf = np.ascontiguousarray(
            x.T.reshape(CT, P, N).transpose(1, 0, 2)).astype(f8)
        xt = np.ascontiguousarray(x.T.reshape(CT, P, N)).astype(bf)
        m = {"xb": xb, "xf": xf, "xt": xt, "wtb": wt, "ng": ng, "aux": aux}
        if auxv is not None:
            m["auxv"] = auxv
        in_maps.append(m)
    return in_maps, diag_one, ln_trivial


def kernel(local_feat, global_feat, pos, W_adj, W_aff, b_aff, ln_gamma,
           ln_beta, **_unused):
    global last_results
    in_maps, diag_one, ln_trivial = _prep_inputs(
        local_feat, W_adj, W_aff, b_aff, ln_gamma, ln_beta)
    nc = _get_program(diag_one, ln_trivial)
    trace = bool(int(os.environ.get("KERNEL_TRACE", "0")))
    res = run_bass_kernel_spmd(nc, in_maps, list(range(B)), trace=trace)
    last_results = res
    out = np.empty((B, N, C), np.float32)
    for bb in range(B):
        yb = np.asarray(res.results[bb]["y"]).astype(np.float32)  # [P, NT, C]
        out[bb] = yb.transpose(1, 0, 2).reshape(N, C)
    return out.reshape(B, T, NN, C)


# revision 5
# speedup vs baseline: 1.2297x; 1.1102x over previous
"""Trainium2 Bass kernel v3 for nn_Diff_prop_18425409699925 (GNN message passing).

Math (per batch element, x = local_feat[b] reshaped to [n=1024, c=256]):
  rno_i = 1/||x_i||;  xn = x*rno;  A = xn xn^T;  G = exp(5A), diag zeroed
  den_i = rowsum(G) = SP_i - e^5;  py = G @ x  (G symmetric -> c-major)
  diff  = x W^T - (py/den) W^T;  y = LeakyReLU(LN(diff))
LN is scale-invariant => y = LeakyReLU(LN(den * xW^T - pyW^T)) exactly:
the 1/den division disappears and den enters only as a per-partition scalar.

v3 pipeline (fixes measured v2 sins):
  - input DMA split across sync + gpsimd queues (v2: one queue, x landed
    ~15us and everything chained after it)
  - >=5us dense PE warmup burst on memset tiles (HAM 1.2->2.4GHz)
  - rno column-broadcast built from 8 [128,128] per-block row-broadcasts
    (DVE tensor_scalar) + 8 PE transposes into one bf16 PSUM tile: no
    SBUF->SBUF DMA hop, no rank-1 matmuls, ~2us after RNO
  - normalization folded ONCE into xn8 = xf * RBC (fp8); the A matmuls and
    exp(scale=5.0, PSUM f32 input) then run exactly like the v1 baseline
    (v2's per-block tmp multiply made the exp chain DVE-paced)
  - tail: pd = xW^T and pu = (py)W^T matmul streams, then one dual-PSUM
    STT per row block D_i = den_i*pd_i - pu_i -> SBUF f32; bn_stats/aggr
    on SBUF; fused Prelu finisher; paired output DMA.  No den broadcast.
"""

import os
import sys

import numpy as np

for _p in ("/opt/trn_rl_repo",):
    if os.path.isdir(_p) and _p not in sys.path:
        sys.path.insert(0, _p)

import ml_dtypes

import concourse.bacc as bacc
import concourse.bass as bass
import concourse.tile as tile
from concourse import mybir
from concourse.bass_utils import run_bass_kernel_spmd

B, T, NN, C = 8, 16, 64, 256
N = T * NN            # 1024 nodes per batch element
P = 128               # partitions
NT = N // P           # 8 n-tiles
CT = C // P           # 2 c-tiles
F32 = mybir.dt.float32
BF16 = mybir.dt.bfloat16
F8 = mybir.dt.float8e4
I32 = mybir.dt.int32
TS = bass.ts
DR = mybir.MatmulPerfMode.DoubleRow

LN_EPS = 1e-5
LEAK = 0.01
E5 = 148.4131591025766          # exp(5): diagonal of exp(5*A) pre-zeroing
MAGIC = float(0x5F3759DF)       # fp32 rsqrt seed constant, as a float


def _emit_rsqrt(nc, sb, out, in_, tmp_tags, n, steps=2):
    """out[:, :n] = 1/sqrt(in_[:, :n]) entirely on DVE (magic seed + Newton
    steps). in_/out fp32 [P, n]. Avoids a second ScalarE table set."""
    mult = mybir.AluOpType.mult
    add = mybir.AluOpType.add
    f = sb.tile([P, n], F32, tag=tmp_tags[0], name="rsq_f")
    ii = sb.tile([P, n], I32, tag=tmp_tags[1], name="rsq_i")
    a = sb.tile([P, n], F32, tag=tmp_tags[2], name="rsq_a")
    c = sb.tile([P, n], F32, tag=tmp_tags[3], name="rsq_c")
    nc.vector.tensor_copy(f[:], in_.bitcast(I32))          # int -> float value
    nc.vector.tensor_scalar(out=f[:], in0=f[:], scalar1=-0.5, scalar2=MAGIC,
                            op0=mult, op1=add)
    nc.vector.tensor_copy(ii[:], f[:])                     # float -> int round
    y = ii.bitcast(F32)
    for _ in range(steps):
        nc.vector.tensor_tensor(out=a[:], in0=y[:], in1=y[:], op=mult)
        nc.vector.scalar_tensor_tensor(out=a[:], in0=a[:], scalar=-0.5,
                                       in1=in_, op0=mult, op1=mult)
        nc.vector.tensor_scalar(out=c[:], in0=a[:], scalar1=1.5, scalar2=None,
                                op0=add)
        nc.vector.tensor_tensor(out=y[:], in0=y[:], in1=c[:], op=mult)
    nc.vector.tensor_copy(out, y[:])


def _build_program(diag_one, ln_trivial, cfg):
    nc = bacc.Bacc("TRN2", target_bir_lowering=False, debug=False)

    xb_d = nc.declare_dram_parameter("xb", [P, NT, C], F8, isOutput=False)
    xf_d = nc.declare_dram_parameter("xf", [P, CT, N], F8, isOutput=False)
    xt_d = nc.declare_dram_parameter("xt", [CT, P, N], BF16, isOutput=False)
    wtb_d = nc.declare_dram_parameter("wtb", [CT, P, C], BF16, isOutput=False)
    ng_d = nc.declare_dram_parameter("ng", [P, P], BF16, isOutput=False)
    aux_d = nc.declare_dram_parameter("aux", [P, 8], F32, isOutput=False)
    if not ln_trivial:
        auxv_d = nc.declare_dram_parameter("auxv", [1, 3 * C], F32,
                                           isOutput=False)
    else:
        auxv_d = None
    y_d = nc.declare_dram_parameter("y", [P, NT, C], BF16, isOutput=True)

    with tile.TileContext(nc) as tc:
        _emit(nc, tc, xb_d, xf_d, xt_d, wtb_d, ng_d, aux_d, auxv_d, y_d,
              diag_one, ln_trivial, cfg)
    nc.finalize()
    return nc


def _emit(nc, tc, xb_d, xf_d, xt_d, wtb_d, ng_d, aux_d, auxv_d, y_d,
          diag_one, ln_trivial, cfg):
    from contextlib import ExitStack

    mult = mybir.AluOpType.mult
    add = mybir.AluOpType.add
    subtract = mybir.AluOpType.subtract
    bypass = mybir.AluOpType.bypass
    amax = mybir.AluOpType.max
    AF = mybir.ActivationFunctionType

    use_prelu = cfg["prelu"]
    n_warm = cfg["warm_mm"]

    with ExitStack() as ctx:
        sb = ctx.enter_context(tc.tile_pool(name="sb", bufs=1))
        ps = ctx.enter_context(tc.tile_pool(name="ps", bufs=1, space="PSUM"))

        # ---------------- SBUF tiles ----------------
        Xb = sb.tile([P, NT, C], F8, tag="Xb", name="Xb")
        xf = sb.tile([P, CT, N], F8, tag="xf", name="xf")
        xn8 = sb.tile([P, CT, N], F8, tag="xn8", name="xn8")
        xtb = sb.tile([P, CT, N], BF16, tag="xtb", name="xtb")
        wtb = sb.tile([P, CT, C], BF16, tag="wtb", name="wtb")
        ng = sb.tile([P, P], BF16, tag="ng", name="ng")
        aux = sb.tile([P, 8], F32, tag="aux", name="aux")
        G8 = sb.tile([P, NT, N], F8, tag="G8", name="G8")
        spyT = sb.tile([P, CT, N], BF16, tag="spyT", name="spyT")
        D = sb.tile([P, NT, C], F32, tag="D", name="D")
        Y = sb.tile([P, NT, C], BF16, tag="Y", name="Y")
        ones1 = sb.tile([1, P], BF16, tag="ones1", name="ones1")
        ON1 = sb.tile([P, P], BF16, tag="ON1", name="ON1")

        SS = sb.tile([P, NT], F32, tag="SS", name="SS")
        RNO = sb.tile([P, NT], F32, tag="RNO", name="RNO")
        SP = sb.tile([P, NT], F32, tag="SP", name="SP")
        SPD = sb.tile([P, NT], F32, tag="SPD", name="SPD")

        BST = sb.tile([P, NT, 6], F32, tag="BST", name="BST")
        MV = sb.tile([P, NT, 2], F32, tag="MV", name="MV")
        VPE = sb.tile([P, NT], F32, tag="VPE", name="VPE")
        RSTD = sb.tile([P, NT], F32, tag="RSTD", name="RSTD")
        NB = sb.tile([P, NT], F32, tag="NB", name="NB")
        warm1 = sb.tile([P, 1], F32, tag="warm1", name="warm1")
        wl = sb.tile([P, P], BF16, tag="wl", name="wl")
        wr = sb.tile([P, 512], BF16, tag="wr", name="wr")

        if not diag_one:
            wdg = sb.tile([P, CT], F32, tag="wdg", name="wdg")
            xn8s = sb.tile([P, CT, N], F8, tag="xn8s", name="xn8s")
        if not ln_trivial:
            auxv = sb.tile([1, 3 * C], F32, tag="auxv", name="auxv")
            bb2 = sb.tile([P, C], F32, tag="bb2", name="bb2")
            gb = sb.tile([P, C], F32, tag="gb", name="gb")
            bb = sb.tile([P, C], F32, tag="bb", name="bb")

        # ------- loads split across BOTH HWDGE queues (sync + scalar; the
        # gpsimd SWDGE queue measured ~6x slower) so x lands ~2x sooner.
        nc.sync.dma_start(Xb[:, 0:2, :], xb_d[:, 0:2, :])
        nc.sync.dma_start(Xb[:, 2:4, :], xb_d[:, 2:4, :])
        nc.sync.dma_start(xf[:, 0, :], xf_d[:, 0, :])
        nc.sync.dma_start(xtb[:, 0, :], xt_d[0, :, :])
        nc.scalar.dma_start(Xb[:, 4:6, :], xb_d[:, 4:6, :])
        nc.scalar.dma_start(Xb[:, 6:8, :], xb_d[:, 6:8, :])
        nc.scalar.dma_start(xf[:, 1, :], xf_d[:, 1, :])
        nc.scalar.dma_start(xtb[:, 1, :], xt_d[1, :, :])
        nc.gpsimd.dma_start(ng[:], ng_d[:])
        nc.gpsimd.dma_start(aux[:], aux_d[:])
        nc.gpsimd.dma_start(wtb[:], wtb_d[:].rearrange("c p d -> p c d"))
        if not ln_trivial:
            nc.gpsimd.dma_start(auxv[:], auxv_d[:])

        # Trigger the single ScalarE table load (exp set) immediately.
        nc.vector.memset(warm1[:], 0.0)
        wscr = sb.tile([P, 1], F32, tag="wscr", name="wscr")
        nc.scalar.activation(wscr[:], warm1[:], AF.Exp)

        # ---------------- PE warmup: dense burst on memset tiles ----------
        # HAM needs ~3.4us of sustained PE busy to un-throttle 1.2->2.4GHz.
        nc.vector.memset(wl[:], 0.0)
        nc.vector.memset(wr[:], 0.0)
        pwarm = ps.tile([P, 512], F32, tag="PY0", name="pwarm")
        for w in range(n_warm):
            nc.tensor.matmul(pwarm[:], wl[:], wr[:], start=True, stop=True)

        nc.vector.memset(ones1[:], 1.0)
        nc.vector.memset(ON1[:], 1.0)
        idb = sb.tile([P, P], BF16, tag="idb", name="idb")
        nc.vector.tensor_scalar(out=idb[:], in0=ng[:], scalar1=-1.0,
                                scalar2=1.0, op0=mult, op1=add)
        if not ln_trivial:
            # b_aff / gamma / beta broadcast to [P, C] on the idle Pool
            nc.gpsimd.partition_broadcast(bb2[:], auxv[0:1, 0:C])
            nc.gpsimd.partition_broadcast(gb[:], auxv[0:1, C:2 * C])
            nc.gpsimd.partition_broadcast(bb[:], auxv[0:1, 2 * C:3 * C])
        if not diag_one:
            nc.vector.tensor_copy(wdg[:], aux[:, 2:4])

        # ---------------- row norms -> RNO ----------------
        # squares split ScalarE/DVE so SS closes right after the last Xb
        # chunk lands (sq accumulates per block; blocks arrive two queues).
        for i in range(NT):
            if i % 2 == 0:
                sqa = sb.tile([P, C], F32, tag="sqa", bufs=2, name="sqa")
                nc.scalar.activation(sqa[:], Xb[:, i, :], AF.Square,
                                     accum_out=SS[:, i:i + 1])
            else:
                sqs = sb.tile([P, C], F32, tag="sqs", bufs=2, name="sqs")
                nc.vector.scalar_tensor_tensor(
                    out=sqs[:], in0=Xb[:, i, :], scalar=1.0, in1=Xb[:, i, :],
                    op0=bypass, op1=mult, accum_out=SS[:, i:i + 1])
        _emit_rsqrt(nc, sb, RNO[:], SS[:], ("rsA", "rsB", "rsC", "rsD"), NT,
                    steps=int(os.environ.get("KERNEL_RNO_STEPS", "1")))

        # ---------------- RBC: rno as columns, via 8 block transposes ------
        # RNOW_b = rno[:, b] broadcast across 128 cols (DVE), transposed by
        # the PE into RBC[:, 128b:128b+128].  bf16 PSUM tile = 1 bank.
        # Two half-tiles in DIFFERENT PSUM banks: the h0 xn8 reads (DVE) run
        # while the PE still writes the h1 transposes -- same-bank PE-W +
        # DVE-R is a fatal HW collision, so the halves must not share one.
        pRBCa = ps.tile([P, 512], BF16, tag="PY0", name="pRBCa")
        pRBCb = ps.tile([P, 512], BF16, tag="PY1", name="pRBCb")
        for b in range(NT):
            RNOW = sb.tile([P, P], BF16, tag="RNOW", bufs=2, name="RNOW")
            nc.vector.tensor_scalar(out=RNOW[:], in0=ON1[:],
                                    scalar1=RNO[:, b:b + 1], scalar2=None,
                                    op0=mult)
            dst = pRBCa if b < 4 else pRBCb
            nc.tensor.transpose(dst[:, TS(b % 4, P)], RNOW[:], idb[:])

        # xn8 = xf * rno_col, reading the broadcast straight from PSUM
        # (one PSUM operand per DVE op is legal; skips two ScalarE copies).
        for h in range(2):
            src = pRBCa if h == 0 else pRBCb
            for cc in range(CT):
                nc.vector.tensor_tensor(
                    out=xn8[:, cc, TS(h, 512)], in0=xf[:, cc, TS(h, 512)],
                    in1=src[:], op=mult)
        if not diag_one:
            for cc in range(CT):
                nc.vector.tensor_scalar(
                    out=xn8s[:, cc, :], in0=xn8[:, cc, :],
                    scalar1=wdg[:, cc:cc + 1], scalar2=None, op0=mult)
            lhs_xn = xn8s
        else:
            lhs_xn = xn8

        # ---------------- A matmuls + exp chain + pyT ----------------
        pa_tiles = {}

        def emit_a_block(k):
            pa_k = ps.tile([P, N], F32, tag=f"A{k % 2}", name=f"pa{k}")
            for j in range(2):
                nc.tensor.matmul(
                    pa_k[:, TS(j, 512)],
                    lhs_xn[:, :, TS(k, P)],
                    xn8[:, :, TS(j, 512)],
                    start=True, stop=True, perf_mode=DR)
            return pa_k

        for k in range(2):
            pa_tiles[k] = emit_a_block(k)

        # pd_i = x_i @ W^T runs DURING the exp window, in the PY banks the
        # pyT accumulation takes over later: pd pair -> plain DVE copy to
        # sd (SBUF) -> bank freed before pyT's first matmul needs it.  The
        # den scale is applied later in the D STT (sd is SBUF, pu is the
        # lone PSUM operand).
        pdA = ps.tile([P, N], F32, tag="PY0", name="pdA")
        pdB = ps.tile([P, N], F32, tag="PY1", name="pdB")
        pd = [pdA[:, TS(i, C)] for i in range(4)] + \
             [pdB[:, TS(i, C)] for i in range(4)]
        sd = sb.tile([P, NT, C], F32, tag="sd", name="sd")

        def emit_pd_pair(pp):
            for i in (2 * pp, 2 * pp + 1):
                for cc in range(CT):
                    nc.tensor.matmul(pd[i], xtb[:, cc, TS(i, P)],
                                     wtb[:, cc, :],
                                     start=(cc == 0), stop=(cc == CT - 1))
            for i in (2 * pp, 2 * pp + 1):
                nc.vector.tensor_copy(sd[:, i, :], pd[i])

        pyT = [ps.tile([P, N], F32, tag=f"PY{k}", name=f"pyT{k}")
               for k in range(CT)]

        for k in range(NT):
            nc.scalar.activation(G8[:, k, :], pa_tiles.pop(k)[:], AF.Exp,
                                 scale=5.0, accum_out=SP[:, k:k + 1])
            # zero the diagonal block of G (it is ~e^5)
            nc.vector.tensor_tensor(out=G8[:, k, TS(k, P)],
                                    in0=G8[:, k, TS(k, P)], in1=ng[:], op=mult)
            if k + 2 < NT:
                pa_tiles[k + 2] = emit_a_block(k + 2)
            if k < 4:
                emit_pd_pair(k)
            if k % 2 == 1:
                # pyT += X_pair^T @ G8_pair  (fp8 DoubleRow over the 2 blocks)
                kp = k // 2
                for cc in range(CT):
                    for j in range(2):
                        nc.tensor.matmul(
                            pyT[cc][:, TS(j, 512)],
                            Xb[:, k - 1:k + 1, TS(cc, P)],
                            G8[:, k - 1:k + 1, TS(j, 512)],
                            start=(kp == 0), stop=(kp == NT // 2 - 1),
                            perf_mode=DR)

        # ---------------- tail ----------------
        # den_i = SP_i - e^5 (per-partition scalar; applied inside the STT)
        nc.vector.tensor_scalar(out=SPD[:], in0=SP[:], scalar1=-E5,
                                scalar2=None, op0=add)

        # spyT = pyT copied to SBUF (PE can't read PSUM); halves split
        # between ScalarE and DVE
        for cc in range(CT):
            for h in range(2):
                if cc == 0:
                    nc.scalar.activation(spyT[:, cc, TS(h, 512)],
                                         pyT[cc][:, TS(h, 512)], AF.Copy)
                else:
                    nc.vector.tensor_copy(spyT[:, cc, TS(h, 512)],
                                          pyT[cc][:, TS(h, 512)])

        # pu_i = py_i @ W^T into the PY banks freed by the spyT copies,
        # then D_i = SPD_i*sd_i - pu_i (single PSUM operand STT).
        puA = ps.tile([P, N], F32, tag="PY0", name="puA")
        puB = ps.tile([P, N], F32, tag="PY1", name="puB")
        pu = [puA[:, TS(i, C)] for i in range(4)] + \
             [puB[:, TS(i, C)] for i in range(4)]
        half = NT // 2

        for pp in range(NT // 2):
            for i in (2 * pp, 2 * pp + 1):
                for cc in range(CT):
                    nc.tensor.matmul(pu[i], spyT[:, cc, TS(i, P)],
                                     wtb[:, cc, :],
                                     start=(cc == 0), stop=(cc == CT - 1))
            for i in (2 * pp, 2 * pp + 1):
                nc.vector.scalar_tensor_tensor(
                    out=D[:, i, :], in0=sd[:, i, :], scalar=SPD[:, i:i + 1],
                    op0=mult, in1=pu[i], op1=subtract)
                if not ln_trivial:
                    # D_i += den_i * b_aff  (row-broadcast bb2, column den)
                    nc.vector.scalar_tensor_tensor(
                        out=D[:, i, :], in0=bb2[:], scalar=SPD[:, i:i + 1],
                        op0=mult, in1=D[:, i, :], op1=add)
                nc.vector.bn_stats(BST[:, i, :], D[:, i, :])
                nc.vector.bn_aggr(MV[:, i, :], BST[:, i, :])

        def emit_finish(h):
            hs = slice(h * half, (h + 1) * half)
            nc.vector.tensor_scalar(out=VPE[:, hs], in0=MV[:, hs, 1],
                                    scalar1=LN_EPS, scalar2=None, op0=add)
            _emit_rsqrt(nc, sb, RSTD[:, hs], VPE[:, hs],
                        ("rsA", "rsB", "rsC", "rsD"), half,
                        steps=int(os.environ.get("KERNEL_RSTD_STEPS", "1")))
            nc.vector.scalar_tensor_tensor(out=NB[:, hs], in0=MV[:, hs, 0],
                                           scalar=-1.0, in1=RSTD[:, hs],
                                           op0=mult, op1=mult)
            for i in range(h * half, (h + 1) * half):
                if ln_trivial and use_prelu and i % 4 != 3:
                    nc.scalar.activation(Y[:, i, :], D[:, i, :], AF.Prelu,
                                         bias=NB[:, i:i + 1],
                                         scale=RSTD[:, i:i + 1], alpha=LEAK)
                else:
                    t = sb.tile([P, C], BF16, tag="fin", bufs=2, name="fin")
                    nc.vector.tensor_scalar(
                        out=t[:], in0=D[:, i, :], scalar1=RSTD[:, i:i + 1],
                        scalar2=NB[:, i:i + 1], op0=mult, op1=add)
                    if not ln_trivial:
                        u2 = sb.tile([P, C], F32, tag="fin2", bufs=2,
                                     name="fin2")
                        nc.vector.scalar_tensor_tensor(
                            out=u2[:], in0=t[:], scalar=1.0, in1=gb[:],
                            op0=bypass, op1=mult)
                        nc.vector.scalar_tensor_tensor(
                            out=t[:], in0=u2[:], scalar=1.0, in1=bb[:],
                            op0=bypass, op1=add)
                    nc.vector.scalar_tensor_tensor(
                        out=Y[:, i, :], in0=t[:], scalar=LEAK, in1=t[:],
                        op0=mult, op1=amax)
                if i % 2 == 1:
                    nc.sync.dma_start(y_d[:, i - 1:i + 1, :],
                                      Y[:, i - 1:i + 1, :])

        emit_finish(0)
        emit_finish(1)


_PROGRAM_CACHE = {}
last_results = None


def _cfg():
    return {
        "prelu": bool(int(os.environ.get("KERNEL_PRELU", "1"))),
        "warm_mm": int(os.environ.get("KERNEL_WARM_MM", "13")),
    }


def _get_program(diag_one=True, ln_trivial=True):
    cfg = _cfg()
    key = (diag_one, ln_trivial, tuple(sorted(cfg.items())))
    if key not in _PROGRAM_CACHE:
        _PROGRAM_CACHE[key] = _build_program(diag_one, ln_trivial, cfg)
    return _PROGRAM_CACHE[key]


def _prep_inputs(local_feat, W_adj, W_aff, b_aff, ln_gamma, ln_beta):
    x_full = np.asarray(local_feat, np.float32).reshape(B, N, C)
    diag = np.ascontiguousarray(np.diagonal(np.asarray(W_adj, np.float32)))
    diag_one = bool(np.all(diag == 1.0))
    g = np.asarray(ln_gamma, np.float32).ravel()
    be = np.asarray(ln_beta, np.float32).ravel()
    b = np.asarray(b_aff, np.float32).ravel()
    ln_trivial = bool(np.all(g == 1.0) and np.all(be == 0.0)
                      and np.all(b == 0.0))

    bf = ml_dtypes.bfloat16
    wt = np.ascontiguousarray(
        np.asarray(W_aff, np.float32).T.reshape(CT, P, C)).astype(bf)
    ng = (1.0 - np.eye(P, dtype=np.float32)).astype(bf)
    aux = np.zeros((P, 8), np.float32)
    if not diag_one:
        aux[:, 2:4] = diag.reshape(CT, P).T
    auxv = None
    if not ln_trivial:
        auxv = np.concatenate([b, g, be]).reshape(1, 3 * C).astype(np.float32)

    f8 = ml_dtypes.float8_e4m3
    in_maps = []
    for bb in range(B):
        x = x_full[bb]
        xb = np.ascontiguousarray(
            x.reshape(NT, P, C).transpose(1, 0, 2)).astype(f8)
        xf = np.ascontiguousarray(
            x.T.reshape(CT, P, N).transpose(1, 0, 2)).astype(f8)
        xt = np.ascontiguousarray(x.T.reshape(CT, P, N)).astype(bf)
        m = {"xb": xb, "xf": xf, "xt": xt, "wtb": wt, "ng": ng,
             "aux": aux}
        if auxv is not None:
            m["auxv"] = auxv
        in_maps.append(m)
    return in_maps, diag_one, ln_trivial


def kernel(local_feat, global_feat, pos, W_adj, W_aff, b_aff, ln_gamma,
           ln_beta, **_unused):
    global last_results
    in_maps, diag_one, ln_trivial = _prep_inputs(
        local_feat, W_adj, W_aff, b_aff, ln_gamma, ln_beta)
    nc = _get_program(diag_one, ln_trivial)
    trace = bool(int(os.environ.get("KERNEL_TRACE", "0")))
    res = run_bass_kernel_spmd(nc, in_maps, list(range(B)), trace=trace)
    last_results = res
    out = np.empty((B, N, C), np.float32)
    for bb in range(B):
        yb = np.asarray(res.results[bb]["y"]).astype(np.float32)  # [P, NT, C]
        out[bb] = yb.transpose(1, 0, 2).reshape(N, C)
    return out.reshape(B, T, NN, C)
